# revision 47
# baseline (speedup 1.0000x reference)
"""GQA dense-transformer block (RMSNorm + QKV + RoPE + causal GQA attention
+ o_proj + residual) on 8 trn2 NeuronCores.

Sharding: 2 (batch) x 4 (head-group tensor parallel). Core c = 4*b + g handles
batch b, q-heads 8g..8g+7, kv-heads 2g..2g+1. Each core produces a partial
o_proj output (feature-major [D, S], bf16); the host sums the 4 partials per
batch and transposes. The RMS-normed residual is included on g==0 cores via
rmsw; g!=0 cores get rmsw=0.

v2 pipeline (vs baseline):
  - per-token scale s computed with one fused DVE sum-of-squares per token
    tile; broadcast across partitions with a diag*s -> ones-matmul trick
    (no DRAM bounce). All Sqrt activations run in the prologue so the ACT
    table set switches exactly once (Sqrt -> Exp).
  - xn = x^T * s computed once (in-place over the x^T tile); feeds Q/K/V
    projections AND the o_proj residual (affine_then_add fuses the
    rms_w-scale + PSUM add).
  - softmax normalize: PV pair accumulates into one [65,1024] PSUM tile;
    denominator row inverted with reciprocal_approx_fast (5x).
  - scheduling: o_proj(w-1) and proj(w+1) matmuls are interleaved into
    attention(w)'s kt loop as "fillers" so the PE never head-of-line
    blocks on the ACT exp chain and stays HAM-warm. PV runs with a
    1-iteration lag behind exp.
"""

import math
import numpy as np

# model dims (hardcoded per contract)
B, S, D = 2, 2048, 2048
HQ, HKV, HD = 32, 8, 64
NC = 8
NG = 4            # head groups
QH = 8            # q heads per core
KH = 2            # kv heads per core
CQ = QH * HD      # 512 q cols per core
W512 = S // 512   # 4 token windows
NT = S // 128     # 16 token tiles
NDC = D // 128    # 16 contraction chunks
PERM = [0, 4, 1, 5, 2, 6, 3, 7]  # local head order: ptile p = (h=p | h=p+4)

_cache = {}
_patched = [False]


def _legalize_bir_bytes(bir):
    """Walrus in this container accepts at most ONE embedded sem-wait per TPB
    instruction ("Too many sync wait commands"). Tile emits several when an
    instruction depends on multiple DMA queues. Split the extras into
    standalone EventSemaphore (pure-wait) instructions on the same engine
    immediately before the instruction — identical blocking semantics."""
    import json
    d = json.loads(bir if isinstance(bir, str) else bir.decode())
    n_split = 0
    stack = [d]
    while stack:
        o = stack.pop()
        if isinstance(o, dict):
            insts = o.get("instructions")
            if isinstance(insts, list) and insts and isinstance(insts[0], dict) \
               and "opcode" in insts[0]:
                new = []
                for inst in insts:
                    si = inst.get("sync_info") or {}
                    ws = si.get("on_wait") or []
                    if len(ws) > 1 and isinstance(inst.get("opcode"), str) \
                       and inst.get("opcode") not in (
                            "EventSemaphore", "UnconditionalBranch",
                            "Call", "ISA"):
                        for k, w in enumerate(ws[:-1]):
                            n_split += 1
                            new.append({
                                "debug": inst.get("debug", 0),
                                "engine": inst["engine"],
                                "ins": [], "outs": [],
                                "name": f"lw{n_split}_{inst['name']}",
                                "opcode": "EventSemaphore",
                                "sync_info": {"on_update": [], "on_wait": [w]},
                            })
                        si["on_wait"] = [ws[-1]]
                    new.append(inst)
                o["instructions"] = new
            else:
                stack.extend(o.values())
        elif isinstance(o, list):
            stack.extend(o)
    return json.dumps(d).encode()


def _install_patch():
    if _patched[0]:
        return
    from concourse import bass_utils as bu
    from concourse import bass2jax as b2j
    orig = bu.compile_bir_kernel

    def patched(bir, *a, **k):
        return orig(_legalize_bir_bytes(bir), *a, **k)

    bu.compile_bir_kernel = patched
    b2j.compile_bir_kernel = patched
    _patched[0] = True


def _build(causal: bool):
    import concourse.bass as bass
    import concourse.mybir as mybir
    from concourse.tile import TileContext

    fp32 = mybir.dt.float32
    bf16 = mybir.dt.bfloat16
    AF = mybir.ActivationFunctionType
    ALU = mybir.AluOpType

    nc = bass.Bass("TRN2")
    xT = nc.dram_tensor("xT", [D, S], bf16, kind="ExternalInput")
    xb_d = nc.dram_tensor("xb", [S, D], bf16, kind="ExternalInput")
    wq_d = nc.dram_tensor("wq", [128, NDC * CQ], bf16, kind="ExternalInput")
    wk_d = nc.dram_tensor("wk", [128, NDC * 128], bf16, kind="ExternalInput")
    wv_d = nc.dram_tensor("wv", [128, NDC * 128], bf16, kind="ExternalInput")
    wo_d = nc.dram_tensor("wo", [128, 4 * D], bf16, kind="ExternalInput")
    cos_d = nc.dram_tensor("cosT", [128, S], bf16, kind="ExternalInput")
    sin_d = nc.dram_tensor("sinT", [128, S], bf16, kind="ExternalInput")
    maskb_d = nc.dram_tensor("maskb", [128, 896], bf16, kind="ExternalInput")
    rmsw_d = nc.dram_tensor("rmsw", [128, NDC], fp32, kind="ExternalInput")
    diag_d = nc.dram_tensor("diag", [128, 128], bf16, kind="ExternalInput")
    out = nc.dram_tensor("out", [D, S], bf16, kind="ExternalOutput")

    with TileContext(nc) as tc:
        with (
            nc.allow_low_precision(reason="bf16 pipeline within 2e-2 tolerance"),
            tc.tile_pool(name="res", bufs=1) as res,
            tc.tile_pool(name="dram", bufs=2, space="DRAM") as dpool,
            tc.tile_pool(name="xn_p", bufs=4) as xn_p,
            tc.tile_pool(name="obw_p", bufs=(2 if causal else 1)) as obw_p,
            tc.tile_pool(name="stat", bufs=2) as sp,
            tc.tile_pool(name="se_p", bufs=1) as se_p,
            tc.tile_pool(name="qt_p", bufs=(3 if causal else 4)) as qt_p,
            tc.tile_pool(name="at_p", bufs=2) as at_p,
            tc.tile_pool(name="rtmp", bufs=1) as rtmp,
            tc.tile_pool(name="bcs_p", bufs=2) as bcs_p,
            tc.tile_pool(name="aex", bufs=3) as aex,
            tc.tile_pool(name="inv_p", bufs=1) as inv_p,
            # PSUM: psc 2 banks + ppv 2x2 + pacc 2 = 8
            tc.tile_pool(name="psc", bufs=1, space="PSUM") as psc,
            tc.tile_pool(name="ppv", bufs=2, space="PSUM") as ppv,
            tc.tile_pool(name="pacc", bufs=2, space="PSUM") as pacc,
        ):
            # ---- resident tiles ----
            KTS = [res.tile([128, 512], bf16, tag=f"kts{w}", name=f"kts{w}")
                   for w in range(W512)]
            VAL = [res.tile([128, 4 * 130], bf16, tag=f"vall{w}", name=f"vall{w}")
                   for w in range(W512)]
            SBC = [res.tile([128, 512], bf16, tag=f"sbc{w}", name=f"sbc{w}")
                   for w in range(W512)]
            cosb = res.tile([128, S], bf16, tag="cosb", name="cosb")
            sinb = res.tile([128, S], bf16, tag="sinb", name="sinb")
            maskb = res.tile([128, 896], bf16, tag="maskb", name="maskb")
            rmswT = res.tile([128, NDC], fp32, tag="rmsw", name="rmswT")
            diag_b = res.tile([128, 128], bf16, tag="diag", name="diag_b")
            wq_r = res.tile([128, NDC * CQ], bf16, tag="wqr", name="wq_r")
            wk_r = res.tile([128, NDC * 128], bf16, tag="wkr", name="wk_r")
            wv_r = res.tile([128, NDC * 128], bf16, tag="wvr", name="wv_r")
            wo_r = res.tile([128, 4 * D], bf16, tag="wor", name="wo_r")
            ones1b = res.tile([1, 128], bf16, tag="ones1b", name="ones1b")
            ones128 = res.tile([128, 128], bf16, tag="ones128", name="ones128")
            epst = res.tile([128, 1], fp32, tag="epst", name="epst")
            ssq = res.tile([128, NT], fp32, tag="ssq", name="ssq")
            sq_all = res.tile([128, NT], fp32, tag="sq", name="sq_all")
            s_all = res.tile([128, NT], fp32, tag="sall", name="s_all")

            nc.vector.memset(ones1b[:, :], 1.0)
            nc.vector.memset(ones128[:, :], 1.0)
            nc.vector.memset(epst[:, :], float(np.finfo(np.float32).eps))
            for w in range(W512):
                nc.vector.memset(VAL[w][:, :], 1.0)
            # DMA issue order matters: queues are FIFO, and the first PE op
            # (psb(0)) transitively needs the stats tiles of window 0 — so
            # those xb loads and the small tables go first; the big x^T /
            # weight walls go after.
            nc.gpsimd.dma_start(out=diag_b[:, :], in_=diag_d[:, :])
            nc.gpsimd.dma_start(out=rmswT[:, :], in_=rmsw_d[:, :])

            xns = {}      # w -> xn tile ([128, NDC*512] bf16, in-place scaled)
            qts = {}      # (w, ct) -> QT tile
            ats = {}      # (w, p) -> AT tile
            fillers = []  # (is_pe, fn) queue

            def pump(n=1):
                # pop until n PE-emitting closures ran (aux closures are free)
                while n > 0 and fillers:
                    is_pe, fn = fillers.pop(0)
                    fn()
                    if is_pe:
                        n -= 1

            def flush():
                while fillers:
                    fillers.pop(0)[1]()

            # ---------------- x load / stats / xn ----------------
            def emit_xtw_load(w):
                t = xn_p.tile([128, NDC * 512], bf16, tag="xn", name="xn")
                wsl = slice(512 * w, 512 * (w + 1))
                nc.gpsimd.dma_start(
                    out=t[:, :].rearrange("p (c t) -> p c t", c=NDC),
                    in_=xT[:, wsl].rearrange("(c p) t -> p c t", p=128))
                xns[w] = t

            def emit_stats(tt, eng=None):
                # ssq[:, tt] = sum over features of x[token,:]^2; the out
                # tensor is a dead scratch (separate from the input so the
                # DVE 2x bf16 mode can engage)
                x_t = sp.tile([128, D], bf16, tag="xs", name="xs")
                nc.gpsimd.dma_start(out=x_t[:, :],
                                    in_=xb_d[tt * 128 : (tt + 1) * 128, :])
                (eng or nc.vector).scalar_tensor_tensor(
                    out=x_t[:, :], in0=x_t[:, :], scalar=1.0, in1=x_t[:, :],
                    op0=ALU.mult, op1=ALU.mult,
                    accum_out=ssq[:, tt : tt + 1])

            def emit_sbc(w, as_filler=False):
                # s = 1/sqrt(mean+eps) for the 4 token tiles of window w,
                # then broadcast across partitions: ones128^T @ (diag * s)
                def body():
                    tsl = slice(4 * w, 4 * (w + 1))
                    nc.scalar.activation(out=sq_all[:, tsl], in_=ssq[:, tsl],
                                         func=AF.Sqrt, bias=epst[:, 0:1],
                                         scale=1.0 / D)
                    nc.vector.reciprocal(out=s_all[:, tsl], in_=sq_all[:, tsl])
                    se = se_p.tile([128, 512], bf16, tag="se", name="se")
                    for vt in range(4):
                        nc.vector.tensor_scalar_mul(
                            se[:, 128 * vt : 128 * (vt + 1)], diag_b[:, :],
                            s_all[:, 4 * w + vt : 4 * w + vt + 1])
                    acc = pacc.tile([128, 512], fp32, tag="acc", name="psb")
                    nc.tensor.matmul(acc[:, :], ones128[:, :], se[:, :],
                                     start=True, stop=True)
                    nc.scalar.copy(out=SBC[w][:, :], in_=acc[:, :])
                if as_filler:
                    fillers.append((True, body))
                else:
                    body()

            def emit_xn(w):
                # xn = x^T * s for the o_proj residual only (projections
                # consume the raw tile). Emitted at iteration w: Tile's WAR
                # tracking orders the in-place writes after all proj reads,
                # and the idle Pool engine absorbs the 1x in-place cost.
                t = xns[w]
                for dc in range(NDC):
                    xsl = slice(512 * dc, 512 * (dc + 1))
                    nc.gpsimd.tensor_mul(t[:, xsl], t[:, xsl], SBC[w][:, :])

            # ---------------- projections + rope (filler-style) -------------
            def emit_rope(w, box, kind):
                # rotate-half RoPE straight off the proj PSUM tile (a
                # PSUM operand exempts the equal-base-partition rule the
                # cross-partition sin reads would otherwise violate).
                # The per-token rms scale s is folded in via sbc at the end
                # (projections consume RAW x^T).
                wsl = slice(512 * w, 512 * (w + 1))
                src = box["t"]
                tmp = rtmp.tile([128, 512], bf16, tag="rt", name="rt")
                for a, bidx in ((0, 1), (1, 0), (2, 3), (3, 2)):
                    nc.vector.tensor_mul(tmp[32 * a : 32 * (a + 1), :],
                                         src[32 * bidx : 32 * (bidx + 1), :],
                                         sinb[32 * a : 32 * (a + 1), wsl])
                t2 = rtmp.tile([128, 512], bf16, tag="rt2", name="rt2")
                nc.vector.tensor_mul(t2[:, :], src[:, :], cosb[:, wsl])
                nc.vector.tensor_add(tmp[:, :], tmp[:, :], t2[:, :])
                if kind == "K":
                    dst = KTS[w]
                else:
                    dst = qt_p.tile([128, 512], bf16, tag=f"qt{kind[1]}",
                                    name=f"qt{kind[1]}")
                    qts[(w, kind[1])] = dst
                nc.vector.tensor_mul(dst[:, :], tmp[:, :], SBC[w][:, :])

            def emit_vall(w, box):
                # V tokens sit on partitions here, so the per-token scale s
                # rides along as the activation's per-partition scale
                vs = box["t"]
                for vt in range(4):
                    nc.scalar.activation(
                        out=VAL[w][:, 130 * vt : 130 * vt + 130]
                            .rearrange("p (h x) -> p h x", h=2)[:, :, 0:64],
                        in_=vs[:, 128 * vt : 128 * (vt + 1)]
                            .rearrange("p (h d) -> p h d", h=2),
                        func=AF.Copy,
                        scale=s_all[:, 4 * w + vt : 4 * w + vt + 1])

            def emit_proj_fillers(w):
                xt = xns[w]
                # K pass: 4 fillers x 4 dc
                kbox = {}
                for blk in range(4):
                    def fk(blk=blk):
                        if blk == 0:
                            kbox["t"] = pacc.tile([128, 512], fp32, tag="acc",
                                                  name="ks")
                        for dc in range(4 * blk, 4 * blk + 4):
                            nc.tensor.matmul(
                                kbox["t"][:, :],
                                wk_r[:, dc * 128 : (dc + 1) * 128],
                                xt[:, 512 * dc : 512 * (dc + 1)],
                                start=(dc == 0), stop=(dc == NDC - 1))
                    fillers.append((True, fk))
                fillers.append((False, lambda: emit_rope(w, kbox, "K")))
                # V pass: 8 fillers x (2 dc x 4 vt)
                vbox = {}
                for blk in range(8):
                    def fv(blk=blk):
                        if blk == 0:
                            vbox["t"] = pacc.tile([128, 512], fp32, tag="acc",
                                                  name="vs")
                        for dc in range(2 * blk, 2 * blk + 2):
                            for vt in range(4):
                                nc.tensor.matmul(
                                    vbox["t"][:, 128 * vt : 128 * (vt + 1)],
                                    xt[:, 512 * dc + 128 * vt : 512 * dc + 128 * (vt + 1)],
                                    wv_r[:, dc * 128 : (dc + 1) * 128],
                                    start=(dc == 0 and vt == 0),
                                    stop=(dc == NDC - 1),
                                    skip_group_check=True)
                    fillers.append((True, fv))
                fillers.append((False, lambda: emit_vall(w, vbox)))
                # Q passes: 4 ct x (4 fillers x 4 dc)
                for ct in range(4):
                    qbox = {}
                    for blk in range(4):
                        def fq(ct=ct, blk=blk, qbox=qbox):
                            if blk == 0:
                                qbox["t"] = pacc.tile([128, 512], fp32,
                                                      tag="acc", name="qs")
                            for dc in range(4 * blk, 4 * blk + 4):
                                nc.tensor.matmul(
                                    qbox["t"][:, :],
                                    wq_r[:, dc * CQ + ct * 128 : dc * CQ + (ct + 1) * 128],
                                    xt[:, 512 * dc : 512 * (dc + 1)],
                                    start=(dc == 0), stop=(dc == NDC - 1))
                        fillers.append((True, fq))
                    fillers.append(
                        (False, lambda w=w, qbox=qbox, ct=ct:
                            emit_rope(w, qbox, ("Q", ct))))

            # ---------------- o_proj + residual (filler-style) --------------
            def emit_oproj_fillers(w):
                obw = obw_p.tile([128, NDC * 512], bf16, tag="obw", name="obw")
                wsl = slice(512 * w, 512 * (w + 1))
                for dc in range(NDC):
                    def fo(dc=dc):
                        pso = pacc.tile([128, 512], fp32, tag="acc", name="pso")
                        for c in range(4):
                            nc.tensor.matmul(
                                pso[:, :],
                                wo_r[:, c * D + dc * 128 : c * D + (dc + 1) * 128],
                                ats[(w, c)][:, :], start=(c == 0), stop=(c == 3))
                        xsl = slice(512 * dc, 512 * (dc + 1))
                        # obw = xn*rmsw + pso in one DVE op
                        nc.vector.scalar_tensor_tensor(
                            out=obw[:, xsl], in0=xns[w][:, xsl],
                            scalar=rmswT[:, dc : dc + 1], in1=pso[:, :],
                            op0=ALU.mult, op1=ALU.add)
                        if dc == NDC - 1:
                            nc.gpsimd.dma_start(
                                out=out[:, wsl].rearrange("(c p) t -> p c t", p=128),
                                in_=obw[:, :].rearrange("p (c t) -> p c t", c=NDC))
                    fillers.append((True, fo))

            # ---------------- attention ----------------
            def emit_attention(w):
                kt_max = 4 * (w + 1) if causal else NT
                pending_norm = [None]
                for p in range(4):
                    at_t = at_p.tile([128, 512], bf16, tag=f"at{p}",
                                     name=f"at{p}")
                    ats[(w, p)] = at_t
                    qt_t = qts[(w, p)]
                    pvs = ppv.tile([65, 1024], fp32, tag="pv", name="pv")
                    exq = {}

                    def pv_pair(kt, pvs=pvs, exq=exq, kt_max=kt_max):
                        ex2 = exq.pop(kt)
                        wv_, lt = kt // 4, kt % 4
                        for h in range(2):
                            nc.tensor.matmul(
                                pvs[:, 512 * h : 512 * (h + 1)],
                                VAL[wv_][:, 130 * lt + 65 * h : 130 * lt + 65 * (h + 1)],
                                ex2[:, 512 * h : 512 * (h + 1)],
                                start=(kt == 0), stop=(kt == kt_max - 1),
                                skip_group_check=True)

                    for kt in range(kt_max):
                        dd = 128 * kt - 512 * w
                        sc2 = psc.tile([128, 1024], fp32, tag="sc", name="sc2")
                        for h in range(2):
                            nc.tensor.matmul(
                                sc2[:, 512 * h : 512 * (h + 1)],
                                KTS[kt // 4][64 * h : 64 * (h + 1),
                                             (kt % 4) * 128 : (kt % 4 + 1) * 128],
                                qt_t[64 * h : 64 * (h + 1), :],
                                start=True, stop=True)
                        ex2 = aex.tile([128, 1024], bf16, tag="ex", name="ex")
                        nc.scalar.activation(out=ex2[:, :], in_=sc2[:, :],
                                             func=AF.Exp)
                        if causal and 0 <= dd <= 384:
                            off = 384 - dd
                            ex2v = ex2[:, :].rearrange("p (a b) -> p a b", a=2)
                            mrep = maskb[:, off : off + 512].rearrange(
                                "p (a f) -> p a f", a=1).to_broadcast((128, 2, 512))
                            nc.vector.tensor_mul(ex2v, ex2v, mrep)
                        exq[kt] = ex2
                        pump(1)
                        if kt == 3 and pending_norm[0] is not None:
                            # previous p's softmax normalize: ~4 kt-cycles
                            # after its last PV, so the (slow) DVE
                            # reciprocal has drained and the bc matmuls
                            # don't stall the PE. PV(p) is unaffected — it
                            # accumulates into its own ppv slot (bufs=2).
                            pending_norm[0]()
                            pending_norm[0] = None
                        if kt >= 1:
                            pv_pair(kt - 1)
                    pump(1)
                    pv_pair(kt_max - 1)
                    # normalize: 1/denominator now; broadcast+apply deferred.
                    # DVE reciprocal is 8 cyc/elem/lane, so a [1,1024] row
                    # costs 5.3us — bounce it through DRAM to spread across
                    # 64 partitions ([64,16] => ~0.4us), then bounce back to
                    # row form for the broadcast matmul. Latency (~5us) is
                    # hidden by the kt==3 deferral of norm_tail.
                    pvrow = inv_p.tile([1, 1024], fp32, tag="pvrow",
                                       name="pvrow")
                    nc.scalar.copy(out=pvrow[:, :], in_=pvs[64:65, :])
                    den_d = dpool.tile([1, 1024], fp32, tag="dend",
                                       name="den_d")
                    nc.gpsimd.dma_start(out=den_d[0:1, :], in_=pvrow[0:1, :])
                    dsp = inv_p.tile([64, 16], fp32, tag="dsp", name="dsp")
                    nc.gpsimd.dma_start(
                        out=dsp[:, :],
                        in_=den_d[0:1, :].rearrange("one (p c) -> (one p) c",
                                                    p=64))
                    ispb = inv_p.tile([64, 16], bf16, tag="ispb", name="ispb")
                    nc.vector.reciprocal(out=ispb[:, :], in_=dsp[:, :])
                    inv_d = dpool.tile([1, 1024], bf16, tag="invd",
                                       name="inv_d")
                    nc.gpsimd.dma_start(
                        out=inv_d[0:1, :].rearrange("one (p c) -> (one p) c",
                                                    p=64),
                        in_=ispb[:, :])
                    # broadcast 1/d across 64 partitions straight from DRAM
                    # (stride-0 source) — no PE matmul, no PSUM tile
                    bcs2 = []
                    for h in range(2):
                        bcs = bcs_p.tile([64, 512], bf16, tag=f"bcs{h}",
                                         name=f"bcs{h}")
                        nc.gpsimd.dma_start(
                            out=bcs[:, :],
                            in_=inv_d[0:1, 512 * h : 512 * (h + 1)]
                                .to_broadcast((64, 512)))
                        bcs2.append(bcs)

                    def norm_tail(pvs=pvs, bcs2=bcs2, at_t=at_t):
                        for h in range(2):
                            nc.vector.tensor_mul(
                                at_t[64 * h : 64 * (h + 1), :],
                                pvs[0:64, 512 * h : 512 * (h + 1)],
                                bcs2[h][:, :])

                    pending_norm[0] = norm_tail
                if pending_norm[0] is not None:
                    pump(3)
                    pending_norm[0]()
                    pending_norm[0] = None

            # ---------------- schedule ----------------
            # prologue: stats for w0/w1 + both their proj passes run dense,
            # so the attention loop is always two windows ahead on proj.
            for tt in range(4):
                emit_stats(tt)
            nc.gpsimd.dma_start(out=wk_r[:, :], in_=wk_d[:, :])
            nc.gpsimd.dma_start(out=wv_r[:, :], in_=wv_d[:, :])
            nc.gpsimd.dma_start(out=cosb[:, :], in_=cos_d[:, :])
            nc.gpsimd.dma_start(out=sinb[:, :], in_=sin_d[:, :])
            emit_xtw_load(0)
            emit_xtw_load(1)
            nc.gpsimd.dma_start(out=wq_r[:, :], in_=wq_d[:, :])
            emit_sbc(0)
            for tt in range(4, 8):
                emit_stats(tt)
            emit_sbc(1)
            emit_proj_fillers(0)
            flush()
            # remaining stats + ALL Sqrt activations must complete in the
            # prologue: a Sqrt pumped mid-attention lands after the Exp
            # table set is resident and computes garbage
            for tt in range(8, NT):
                emit_stats(tt)
            emit_proj_fillers(1)
            flush()
            emit_sbc(2)
            emit_sbc(3)
            nc.gpsimd.dma_start(out=wo_r[:, :], in_=wo_d[:, :])
            nc.gpsimd.dma_start(out=maskb[:, :], in_=maskb_d[:, :])
            emit_xtw_load(2)

            if causal:
                for w in range(W512):
                    if w == 0:
                        emit_xtw_load(3)
                    emit_xn(w)
                    if w > 0:
                        emit_oproj_fillers(w - 1)
                    if w + 2 < W512:
                        emit_proj_fillers(w + 2)
                    emit_attention(w)
                    flush()
                emit_oproj_fillers(W512 - 1)
                flush()
            else:
                # all K/V (and Q) must exist before any attention window
                emit_xtw_load(3)
                for w in range(2, W512):
                    emit_proj_fillers(w)
                    flush()
                for w in range(W512):
                    emit_xn(w)
                    if w > 0:
                        emit_oproj_fillers(w - 1)
                    emit_attention(w)
                    flush()
                emit_oproj_fillers(W512 - 1)
                flush()
    return nc


def _host_prep(x, rms_w, Wq, Wk, Wv, Wo):
    import ml_dtypes
    f32 = np.float32
    bf16 = ml_dtypes.bfloat16
    x = np.asarray(x, f32)
    rms_w = np.asarray(rms_w, f32)
    wq_full = (np.asarray(Wq, f32) * rms_w[:, None] / math.sqrt(HD)).astype(f32)
    wk_full = (np.asarray(Wk, f32) * rms_w[:, None]).astype(f32)
    wv_full = (np.asarray(Wv, f32) * rms_w[:, None]).astype(f32)
    Wo = np.asarray(Wo, f32)

    inv_f = (1.0 / (10000.0 ** (np.arange(0, HD, 2, dtype=f32) / HD))).astype(f32)
    freqs = np.arange(S, dtype=f32)[:, None] * inv_f[None, :]   # [S, 32]
    cos = np.cos(freqs).astype(f32).T                           # [32, S]
    sin = np.sin(freqs).astype(f32).T
    cosT = np.tile(np.concatenate([cos, cos], 0), (2, 1))       # [128, S]
    sinT = np.tile(np.concatenate([-sin, sin], 0), (2, 1))

    kk = np.arange(128)[:, None]
    jj = np.arange(896)[None, :]
    maskb = (jj >= kk + 384).astype(f32)
    diag = np.eye(128, dtype=f32)

    per_core = []
    for c in range(NC):
        b, g = c // 4, c % 4
        heads = [8 * g + h for h in PERM]
        wq_g = np.ascontiguousarray(
            np.concatenate([wq_full[:, 64 * h : 64 * (h + 1)] for h in heads], axis=1))
        wo_g = np.ascontiguousarray(
            np.concatenate([Wo[64 * h : 64 * (h + 1), :] for h in heads], axis=0))
        wk_g = np.ascontiguousarray(wk_full[:, 128 * g : 128 * (g + 1)])
        wv_g = np.ascontiguousarray(wv_full[:, 128 * g : 128 * (g + 1)])
        # chunk-major resident layouts: [128, chunk-index * cols]
        wq_r = np.ascontiguousarray(
            wq_g.reshape(NDC, 128, CQ).transpose(1, 0, 2).reshape(128, NDC * CQ))
        wk_r = np.ascontiguousarray(
            wk_g.reshape(NDC, 128, 128).transpose(1, 0, 2).reshape(128, NDC * 128))
        wv_r = np.ascontiguousarray(
            wv_g.reshape(NDC, 128, 128).transpose(1, 0, 2).reshape(128, NDC * 128))
        wo_r = np.ascontiguousarray(
            wo_g.reshape(4, 128, D).transpose(1, 0, 2).reshape(128, 4 * D))
        rmsw_g = rms_w if g == 0 else np.zeros((D,), f32)
        rmswT = np.ascontiguousarray(
            rmsw_g.reshape(NDC, 128).T.astype(f32))        # [128, NDC]
        xb = x[b].astype(bf16)
        per_core.append({
            "xT": np.ascontiguousarray(xb.T),
            "xb": np.ascontiguousarray(xb),
            "wq": wq_r.astype(bf16), "wk": wk_r.astype(bf16),
            "wv": wv_r.astype(bf16), "wo": wo_r.astype(bf16),
            "cosT": np.ascontiguousarray(cosT.astype(bf16)),
            "sinT": np.ascontiguousarray(sinT.astype(bf16)),
            "maskb": maskb.astype(bf16), "rmsw": rmswT,
            "diag": diag.astype(bf16),
        })
    return per_core


def kernel(x, rms_w, Wq, Wk, Wv, Wo, apply_causal_mask, _trace=False):
    from concourse import bass_utils
    _install_patch()
    causal = bool(int(np.asarray(apply_causal_mask)))
    if causal not in _cache:
        _cache[causal] = _build(causal)
    nc = _cache[causal]
    in_maps = _host_prep(x, rms_w, Wq, Wk, Wv, Wo)
    r = bass_utils.run_bass_kernel_spmd(nc, in_maps, core_ids=list(range(NC)),
                                        trace=_trace)
    outs = [np.asarray(r.results[c]["out"], dtype=np.float32) for c in range(NC)]
    full = np.stack([(outs[4 * b] + outs[4 * b + 1] + outs[4 * b + 2] + outs[4 * b + 3]).T
                     for b in range(B)]).astype(np.float32)
    if _trace:
        kernel.last_exec_time_ns = r.exec_time_ns
        kernel.last_result = r
    return full


# revision 54
# speedup vs baseline: 1.0093x; 1.0093x over previous
"""GQA dense-transformer block (RMSNorm + QKV + RoPE + causal GQA attention
+ o_proj + residual) on 8 trn2 NeuronCores.

Sharding: 2 (batch) x 4 (head-group tensor parallel). Core c = 4*b + g handles
batch b, q-heads 8g..8g+7, kv-heads 2g..2g+1. Each core produces a partial
o_proj output (feature-major [D, S], bf16); the host sums the 4 partials per
batch and transposes. The RMS-normed residual is included on g==0 cores via
rmsw; g!=0 cores get rmsw=0.

v2 pipeline (vs baseline):
  - per-token scale s computed with one fused DVE sum-of-squares per token
    tile; broadcast across partitions with a diag*s -> ones-matmul trick
    (no DRAM bounce). All Sqrt activations run in the prologue so the ACT
    table set switches exactly once (Sqrt -> Exp).
  - xn = x^T * s computed once (in-place over the x^T tile); feeds Q/K/V
    projections AND the o_proj residual (affine_then_add fuses the
    rms_w-scale + PSUM add).
  - softmax normalize: PV pair accumulates into one [65,1024] PSUM tile;
    denominator row inverted with reciprocal_approx_fast (5x).
  - scheduling: o_proj(w-1) and proj(w+1) matmuls are interleaved into
    attention(w)'s kt loop as "fillers" so the PE never head-of-line
    blocks on the ACT exp chain and stays HAM-warm. PV runs with a
    1-iteration lag behind exp.
"""

import math
import numpy as np

# model dims (hardcoded per contract)
B, S, D = 2, 2048, 2048
HQ, HKV, HD = 32, 8, 64
NC = 8
NG = 4            # head groups
QH = 8            # q heads per core
KH = 2            # kv heads per core
CQ = QH * HD      # 512 q cols per core
W512 = S // 512   # 4 token windows
NT = S // 128     # 16 token tiles
NDC = D // 128    # 16 contraction chunks
PERM = [0, 4, 1, 5, 2, 6, 3, 7]  # local head order: ptile p = (h=p | h=p+4)

_cache = {}
_patched = [False]


def _legalize_bir_bytes(bir):
    """Walrus in this container accepts at most ONE embedded sem-wait per TPB
    instruction ("Too many sync wait commands"). Tile emits several when an
    instruction depends on multiple DMA queues. Split the extras into
    standalone EventSemaphore (pure-wait) instructions on the same engine
    immediately before the instruction — identical blocking semantics."""
    import json
    d = json.loads(bir if isinstance(bir, str) else bir.decode())
    n_split = 0
    stack = [d]
    while stack:
        o = stack.pop()
        if isinstance(o, dict):
            insts = o.get("instructions")
            if isinstance(insts, list) and insts and isinstance(insts[0], dict) \
               and "opcode" in insts[0]:
                new = []
                for inst in insts:
                    si = inst.get("sync_info") or {}
                    ws = si.get("on_wait") or []
                    if len(ws) > 1 and isinstance(inst.get("opcode"), str) \
                       and inst.get("opcode") not in (
                            "EventSemaphore", "UnconditionalBranch",
                            "Call", "ISA"):
                        for k, w in enumerate(ws[:-1]):
                            n_split += 1
                            new.append({
                                "debug": inst.get("debug", 0),
                                "engine": inst["engine"],
                                "ins": [], "outs": [],
                                "name": f"lw{n_split}_{inst['name']}",
                                "opcode": "EventSemaphore",
                                "sync_info": {"on_update": [], "on_wait": [w]},
                            })
                        si["on_wait"] = [ws[-1]]
                    new.append(inst)
                o["instructions"] = new
            else:
                stack.extend(o.values())
        elif isinstance(o, list):
            stack.extend(o)
    return json.dumps(d).encode()


def _install_patch():
    if _patched[0]:
        return
    from concourse import bass_utils as bu
    from concourse import bass2jax as b2j
    orig = bu.compile_bir_kernel

    def patched(bir, *a, **k):
        return orig(_legalize_bir_bytes(bir), *a, **k)

    bu.compile_bir_kernel = patched
    b2j.compile_bir_kernel = patched
    _patched[0] = True


def _build(causal: bool):
    import concourse.bass as bass
    import concourse.mybir as mybir
    from concourse.tile import TileContext

    fp32 = mybir.dt.float32
    bf16 = mybir.dt.bfloat16
    AF = mybir.ActivationFunctionType
    ALU = mybir.AluOpType

    nc = bass.Bass("TRN2")
    # x^T and out are stored window-major + chunk-major per partition so the
    # per-window DMA is one contiguous 16KB run per partition (128
    # descriptors instead of 2048)
    xT = nc.dram_tensor("xT", [128, W512 * NDC * 512], bf16,
                        kind="ExternalInput")
    xb_d = nc.dram_tensor("xb", [S, D], bf16, kind="ExternalInput")
    wq_d = nc.dram_tensor("wq", [128, NDC * CQ], bf16, kind="ExternalInput")
    wk_d = nc.dram_tensor("wk", [128, NDC * 128], bf16, kind="ExternalInput")
    wv_d = nc.dram_tensor("wv", [128, NDC * 128], bf16, kind="ExternalInput")
    wo_d = nc.dram_tensor("wo", [128, 4 * D], bf16, kind="ExternalInput")
    cos_d = nc.dram_tensor("cosT", [128, S], bf16, kind="ExternalInput")
    sin_d = nc.dram_tensor("sinT", [128, S], bf16, kind="ExternalInput")
    maskb_d = nc.dram_tensor("maskb", [128, 896], bf16, kind="ExternalInput")
    rmsw_d = nc.dram_tensor("rmsw", [128, NDC], fp32, kind="ExternalInput")
    diag_d = nc.dram_tensor("diag", [128, 128], bf16, kind="ExternalInput")
    out = nc.dram_tensor("out", [128, W512 * NDC * 512], bf16,
                         kind="ExternalOutput")

    with TileContext(nc) as tc:
        with (
            nc.allow_low_precision(reason="bf16 pipeline within 2e-2 tolerance"),
            tc.tile_pool(name="res", bufs=1) as res,
            tc.tile_pool(name="dram", bufs=2, space="DRAM") as dpool,
            tc.tile_pool(name="xn_p", bufs=4) as xn_p,
            tc.tile_pool(name="obw_p", bufs=(2 if causal else 1)) as obw_p,
            tc.tile_pool(name="stat", bufs=2) as sp,
            tc.tile_pool(name="se_p", bufs=1) as se_p,
            tc.tile_pool(name="qt_p", bufs=(3 if causal else 4)) as qt_p,
            tc.tile_pool(name="at_p", bufs=2) as at_p,
            tc.tile_pool(name="rtmp", bufs=1) as rtmp,
            tc.tile_pool(name="bcs_p", bufs=2) as bcs_p,
            tc.tile_pool(name="aex", bufs=3) as aex,
            tc.tile_pool(name="inv_p", bufs=1) as inv_p,
            # PSUM: psc 2 banks + ppv 2x2 + pacc 2 = 8
            tc.tile_pool(name="psc", bufs=1, space="PSUM") as psc,
            tc.tile_pool(name="ppv", bufs=2, space="PSUM") as ppv,
            tc.tile_pool(name="pacc", bufs=2, space="PSUM") as pacc,
        ):
            # ---- resident tiles ----
            KTS = [res.tile([128, 512], bf16, tag=f"kts{w}", name=f"kts{w}")
                   for w in range(W512)]
            VAL = [res.tile([128, 4 * 130], bf16, tag=f"vall{w}", name=f"vall{w}")
                   for w in range(W512)]
            SBC = [res.tile([128, 512], bf16, tag=f"sbc{w}", name=f"sbc{w}")
                   for w in range(W512)]
            cosb = res.tile([128, S], bf16, tag="cosb", name="cosb")
            sinb = res.tile([128, S], bf16, tag="sinb", name="sinb")
            maskb = res.tile([128, 896], bf16, tag="maskb", name="maskb")
            rmswT = res.tile([128, NDC], fp32, tag="rmsw", name="rmswT")
            diag_b = res.tile([128, 128], bf16, tag="diag", name="diag_b")
            wq_r = res.tile([128, NDC * CQ], bf16, tag="wqr", name="wq_r")
            wk_r = res.tile([128, NDC * 128], bf16, tag="wkr", name="wk_r")
            wv_r = res.tile([128, NDC * 128], bf16, tag="wvr", name="wv_r")
            wo_r = res.tile([128, 4 * D], bf16, tag="wor", name="wo_r")
            ones1b = res.tile([1, 128], bf16, tag="ones1b", name="ones1b")
            ones128 = res.tile([128, 128], bf16, tag="ones128", name="ones128")
            epst = res.tile([128, 1], fp32, tag="epst", name="epst")
            ssq = res.tile([128, NT], fp32, tag="ssq", name="ssq")
            sq_all = res.tile([128, NT], fp32, tag="sq", name="sq_all")
            s_all = res.tile([128, NT], fp32, tag="sall", name="s_all")

            nc.vector.memset(ones1b[:, :], 1.0)
            nc.vector.memset(ones128[:, :], 1.0)
            nc.vector.memset(epst[:, :], float(np.finfo(np.float32).eps))
            for w in range(W512):
                nc.vector.memset(VAL[w][:, :], 1.0)
            # DMA issue order matters: queues are FIFO, and the first PE op
            # (psb(0)) transitively needs the stats tiles of window 0 — so
            # those xb loads and the small tables go first; the big x^T /
            # weight walls go after.
            nc.gpsimd.dma_start(out=diag_b[:, :], in_=diag_d[:, :])
            nc.gpsimd.dma_start(out=rmswT[:, :], in_=rmsw_d[:, :])

            xns = {}      # w -> xn tile ([128, NDC*512] bf16, in-place scaled)
            qts = {}      # (w, ct) -> QT tile
            ats = {}      # (w, p) -> AT tile
            fillers = []  # (is_pe, fn) queue

            def pump(n=1):
                # pop until n PE-emitting closures ran (aux closures are free)
                while n > 0 and fillers:
                    is_pe, fn = fillers.pop(0)
                    fn()
                    if is_pe:
                        n -= 1

            def flush():
                while fillers:
                    fillers.pop(0)[1]()

            # ---------------- x load / stats / xn ----------------
            def emit_xtw_load(w):
                t = xn_p.tile([128, NDC * 512], bf16, tag="xn", name="xn")
                nc.gpsimd.dma_start(
                    out=t[:, :],
                    in_=xT[:, NDC * 512 * w : NDC * 512 * (w + 1)])
                xns[w] = t

            def emit_stats(tt, eng=None):
                # ssq[:, tt] = sum over features of x[token,:]^2; the out
                # tensor is a dead scratch (separate from the input so the
                # DVE 2x bf16 mode can engage)
                x_t = sp.tile([128, D], bf16, tag="xs", name="xs")
                nc.gpsimd.dma_start(out=x_t[:, :],
                                    in_=xb_d[tt * 128 : (tt + 1) * 128, :])
                (eng or nc.vector).scalar_tensor_tensor(
                    out=x_t[:, :], in0=x_t[:, :], scalar=1.0, in1=x_t[:, :],
                    op0=ALU.mult, op1=ALU.mult,
                    accum_out=ssq[:, tt : tt + 1])

            def emit_sbc(w, as_filler=False):
                # s = 1/sqrt(mean+eps) for the 4 token tiles of window w,
                # then broadcast across partitions: ones128^T @ (diag * s)
                def body():
                    tsl = slice(4 * w, 4 * (w + 1))
                    nc.scalar.activation(out=sq_all[:, tsl], in_=ssq[:, tsl],
                                         func=AF.Sqrt, bias=epst[:, 0:1],
                                         scale=1.0 / D)
                    nc.vector.reciprocal(out=s_all[:, tsl], in_=sq_all[:, tsl])
                    se = se_p.tile([128, 512], bf16, tag="se", name="se")
                    for vt in range(4):
                        nc.vector.tensor_scalar_mul(
                            se[:, 128 * vt : 128 * (vt + 1)], diag_b[:, :],
                            s_all[:, 4 * w + vt : 4 * w + vt + 1])
                    acc = pacc.tile([128, 512], fp32, tag="acc", name="psb")
                    nc.tensor.matmul(acc[:, :], ones128[:, :], se[:, :],
                                     start=True, stop=True)
                    nc.scalar.copy(out=SBC[w][:, :], in_=acc[:, :])
                if as_filler:
                    fillers.append((True, body))
                else:
                    body()

            def emit_xn(w):
                # xn = x^T * s for the o_proj residual only (projections
                # consume the raw tile). Emitted at iteration w: Tile's WAR
                # tracking orders the in-place writes after all proj reads,
                # and the idle Pool engine absorbs the 1x in-place cost.
                t = xns[w]
                for dc in range(NDC):
                    xsl = slice(512 * dc, 512 * (dc + 1))
                    nc.gpsimd.tensor_mul(t[:, xsl], t[:, xsl], SBC[w][:, :])

            # ---------------- projections + rope (filler-style) -------------
            def emit_rope(w, box, kind):
                # rotate-half RoPE straight off the proj PSUM tile (a
                # PSUM operand exempts the equal-base-partition rule the
                # cross-partition sin reads would otherwise violate).
                # The per-token rms scale s is folded in via sbc at the end
                # (projections consume RAW x^T).
                wsl = slice(512 * w, 512 * (w + 1))
                src = box["t"]
                tmp = rtmp.tile([128, 512], bf16, tag="rt", name="rt")
                for a, bidx in ((0, 1), (1, 0), (2, 3), (3, 2)):
                    nc.vector.tensor_mul(tmp[32 * a : 32 * (a + 1), :],
                                         src[32 * bidx : 32 * (bidx + 1), :],
                                         sinb[32 * a : 32 * (a + 1), wsl])
                t2 = rtmp.tile([128, 512], bf16, tag="rt2", name="rt2")
                nc.vector.tensor_mul(t2[:, :], src[:, :], cosb[:, wsl])
                nc.vector.tensor_add(tmp[:, :], tmp[:, :], t2[:, :])
                if kind == "K":
                    dst = KTS[w]
                else:
                    dst = qt_p.tile([128, 512], bf16, tag=f"qt{kind[1]}",
                                    name=f"qt{kind[1]}")
                    qts[(w, kind[1])] = dst
                nc.vector.tensor_mul(dst[:, :], tmp[:, :], SBC[w][:, :])

            def emit_vall(w, box):
                # V tokens sit on partitions here, so the per-token scale s
                # rides along as the activation's per-partition scale
                vs = box["t"]
                for vt in range(4):
                    nc.scalar.activation(
                        out=VAL[w][:, 130 * vt : 130 * vt + 130]
                            .rearrange("p (h x) -> p h x", h=2)[:, :, 0:64],
                        in_=vs[:, 128 * vt : 128 * (vt + 1)]
                            .rearrange("p (h d) -> p h d", h=2),
                        func=AF.Copy,
                        scale=s_all[:, 4 * w + vt : 4 * w + vt + 1])

            def emit_proj_fillers(w):
                xt = xns[w]
                # K pass: 4 fillers x 4 dc
                kbox = {}
                for blk in range(4):
                    def fk(blk=blk):
                        if blk == 0:
                            kbox["t"] = pacc.tile([128, 512], fp32, tag="acc",
                                                  name="ks")
                        for dc in range(4 * blk, 4 * blk + 4):
                            nc.tensor.matmul(
                                kbox["t"][:, :],
                                wk_r[:, dc * 128 : (dc + 1) * 128],
                                xt[:, 512 * dc : 512 * (dc + 1)],
                                start=(dc == 0), stop=(dc == NDC - 1))
                    fillers.append((True, fk))
                fillers.append((False, lambda: emit_rope(w, kbox, "K")))
                # V pass: 8 fillers x (2 dc x 4 vt)
                vbox = {}
                for blk in range(8):
                    def fv(blk=blk):
                        if blk == 0:
                            vbox["t"] = pacc.tile([128, 512], fp32, tag="acc",
                                                  name="vs")
                        for dc in range(2 * blk, 2 * blk + 2):
                            for vt in range(4):
                                nc.tensor.matmul(
                                    vbox["t"][:, 128 * vt : 128 * (vt + 1)],
                                    xt[:, 512 * dc + 128 * vt : 512 * dc + 128 * (vt + 1)],
                                    wv_r[:, dc * 128 : (dc + 1) * 128],
                                    start=(dc == 0 and vt == 0),
                                    stop=(dc == NDC - 1),
                                    skip_group_check=True)
                    fillers.append((True, fv))
                fillers.append((False, lambda: emit_vall(w, vbox)))
                # Q passes: 4 ct x (4 fillers x 4 dc)
                for ct in range(4):
                    qbox = {}
                    for blk in range(4):
                        def fq(ct=ct, blk=blk, qbox=qbox):
                            if blk == 0:
                                qbox["t"] = pacc.tile([128, 512], fp32,
                                                      tag="acc", name="qs")
                            for dc in range(4 * blk, 4 * blk + 4):
                                nc.tensor.matmul(
                                    qbox["t"][:, :],
                                    wq_r[:, dc * CQ + ct * 128 : dc * CQ + (ct + 1) * 128],
                                    xt[:, 512 * dc : 512 * (dc + 1)],
                                    start=(dc == 0), stop=(dc == NDC - 1))
                        fillers.append((True, fq))
                    fillers.append(
                        (False, lambda w=w, qbox=qbox, ct=ct:
                            emit_rope(w, qbox, ("Q", ct))))

            # ---------------- o_proj + residual (filler-style) --------------
            def emit_oproj_fillers(w):
                obw = obw_p.tile([128, NDC * 512], bf16, tag="obw", name="obw")
                wsl = slice(512 * w, 512 * (w + 1))
                for dc in range(NDC):
                    def fo(dc=dc):
                        pso = pacc.tile([128, 512], fp32, tag="acc", name="pso")
                        for c in range(4):
                            nc.tensor.matmul(
                                pso[:, :],
                                wo_r[:, c * D + dc * 128 : c * D + (dc + 1) * 128],
                                ats[(w, c)][:, :], start=(c == 0), stop=(c == 3))
                        xsl = slice(512 * dc, 512 * (dc + 1))
                        # obw = xn*rmsw + pso in one DVE op
                        nc.vector.scalar_tensor_tensor(
                            out=obw[:, xsl], in0=xns[w][:, xsl],
                            scalar=rmswT[:, dc : dc + 1], in1=pso[:, :],
                            op0=ALU.mult, op1=ALU.add)
                        if dc == NDC - 1:
                            nc.gpsimd.dma_start(
                                out=out[:, NDC * 512 * w : NDC * 512 * (w + 1)],
                                in_=obw[:, :])
                    fillers.append((True, fo))

            # ---------------- attention ----------------
            def emit_attention(w):
                kt_max = 4 * (w + 1) if causal else NT
                pending_norm = [None]
                for p in range(4):
                    at_t = at_p.tile([128, 512], bf16, tag=f"at{p}",
                                     name=f"at{p}")
                    ats[(w, p)] = at_t
                    qt_t = qts[(w, p)]
                    pvs = ppv.tile([65, 1024], fp32, tag="pv", name="pv")
                    exq = {}

                    def pv_pair(kt, pvs=pvs, exq=exq, kt_max=kt_max):
                        ex2 = exq.pop(kt)
                        wv_, lt = kt // 4, kt % 4
                        for h in range(2):
                            nc.tensor.matmul(
                                pvs[:, 512 * h : 512 * (h + 1)],
                                VAL[wv_][:, 130 * lt + 65 * h : 130 * lt + 65 * (h + 1)],
                                ex2[:, 512 * h : 512 * (h + 1)],
                                start=(kt == 0), stop=(kt == kt_max - 1),
                                skip_group_check=True)

                    for kt in range(kt_max):
                        dd = 128 * kt - 512 * w
                        sc2 = psc.tile([128, 1024], fp32, tag="sc", name="sc2")
                        for h in range(2):
                            nc.tensor.matmul(
                                sc2[:, 512 * h : 512 * (h + 1)],
                                KTS[kt // 4][64 * h : 64 * (h + 1),
                                             (kt % 4) * 128 : (kt % 4 + 1) * 128],
                                qt_t[64 * h : 64 * (h + 1), :],
                                start=True, stop=True)
                        ex2 = aex.tile([128, 1024], bf16, tag="ex", name="ex")
                        nc.scalar.activation(out=ex2[:, :], in_=sc2[:, :],
                                             func=AF.Exp)
                        if causal and 0 <= dd <= 384:
                            off = 384 - dd
                            ex2v = ex2[:, :].rearrange("p (a b) -> p a b", a=2)
                            mrep = maskb[:, off : off + 512].rearrange(
                                "p (a f) -> p a f", a=1).to_broadcast((128, 2, 512))
                            nc.vector.tensor_mul(ex2v, ex2v, mrep)
                        exq[kt] = ex2
                        pump(1)
                        if kt == 3 and pending_norm[0] is not None:
                            # previous p's softmax normalize: ~4 kt-cycles
                            # after its last PV, so the (slow) DVE
                            # reciprocal has drained and the bc matmuls
                            # don't stall the PE. PV(p) is unaffected — it
                            # accumulates into its own ppv slot (bufs=2).
                            pending_norm[0]()
                            pending_norm[0] = None
                        if kt >= 1:
                            pv_pair(kt - 1)
                    pump(1)
                    pv_pair(kt_max - 1)
                    # normalize: 1/denominator now; broadcast+apply deferred.
                    # DVE reciprocal is 8 cyc/elem/lane, so a [1,1024] row
                    # costs 5.3us — bounce it through DRAM to spread across
                    # 64 partitions ([64,16] => ~0.4us), then bounce back to
                    # row form for the broadcast matmul. Latency (~5us) is
                    # hidden by the kt==3 deferral of norm_tail.
                    pvrow = inv_p.tile([1, 1024], fp32, tag="pvrow",
                                       name="pvrow")
                    nc.scalar.copy(out=pvrow[:, :], in_=pvs[64:65, :])
                    den_d = dpool.tile([1, 1024], fp32, tag="dend",
                                       name="den_d")
                    nc.gpsimd.dma_start(out=den_d[0:1, :], in_=pvrow[0:1, :])
                    dsp = inv_p.tile([64, 16], fp32, tag="dsp", name="dsp")
                    nc.gpsimd.dma_start(
                        out=dsp[:, :],
                        in_=den_d[0:1, :].rearrange("one (p c) -> (one p) c",
                                                    p=64))
                    ispb = inv_p.tile([64, 16], bf16, tag="ispb", name="ispb")
                    nc.vector.reciprocal(out=ispb[:, :], in_=dsp[:, :])
                    # plain write AP (rearranged DRAM write APs have been
                    # seen to race with their readers); reads below use a
                    # row view of the same [64,16] region (t = 16p + c)
                    inv_d = dpool.tile([64, 16], bf16, tag="invd",
                                       name="inv_d")
                    nc.gpsimd.dma_start(out=inv_d[:, :], in_=ispb[:, :])
                    inv_row = inv_d[:, :].rearrange("p c -> (p c)") \
                        .rearrange("(one t) -> one t", one=1)
                    # broadcast 1/d across 64 partitions straight from DRAM
                    # (stride-0 source) — no PE matmul, no PSUM tile
                    bcs2 = []
                    for h in range(2):
                        bcs = bcs_p.tile([64, 512], bf16, tag=f"bcs{h}",
                                         name=f"bcs{h}")
                        nc.gpsimd.dma_start(
                            out=bcs[:, :],
                            in_=inv_row[0:1, 512 * h : 512 * (h + 1)]
                                .to_broadcast((64, 512)))
                        bcs2.append(bcs)

                    def norm_tail(pvs=pvs, bcs2=bcs2, at_t=at_t):
                        for h in range(2):
                            nc.vector.tensor_mul(
                                at_t[64 * h : 64 * (h + 1), :],
                                pvs[0:64, 512 * h : 512 * (h + 1)],
                                bcs2[h][:, :])

                    pending_norm[0] = norm_tail
                if pending_norm[0] is not None:
                    pump(3)
                    pending_norm[0]()
                    pending_norm[0] = None

            # ---------------- schedule ----------------
            # prologue: stats for w0/w1 + both their proj passes run dense,
            # so the attention loop is always two windows ahead on proj.
            for tt in range(4):
                emit_stats(tt)
            nc.gpsimd.dma_start(out=wk_r[:, :], in_=wk_d[:, :])
            nc.gpsimd.dma_start(out=wv_r[:, :], in_=wv_d[:, :])
            nc.gpsimd.dma_start(out=cosb[:, :], in_=cos_d[:, :])
            nc.gpsimd.dma_start(out=sinb[:, :], in_=sin_d[:, :])
            emit_xtw_load(0)
            emit_xtw_load(1)
            nc.gpsimd.dma_start(out=wq_r[:, :], in_=wq_d[:, :])
            emit_sbc(0)
            for tt in range(4, 8):
                emit_stats(tt)
            emit_sbc(1)
            emit_proj_fillers(0)
            flush()
            # remaining stats + ALL Sqrt activations must complete in the
            # prologue: a Sqrt pumped mid-attention lands after the Exp
            # table set is resident and computes garbage
            for tt in range(8, NT):
                emit_stats(tt)
            emit_proj_fillers(1)
            flush()
            emit_sbc(2)
            emit_sbc(3)
            nc.gpsimd.dma_start(out=wo_r[:, :], in_=wo_d[:, :])
            nc.gpsimd.dma_start(out=maskb[:, :], in_=maskb_d[:, :])
            emit_xtw_load(2)

            if causal:
                for w in range(W512):
                    if w == 0:
                        emit_xtw_load(3)
                    emit_xn(w)
                    if w > 0:
                        emit_oproj_fillers(w - 1)
                    if w + 2 < W512:
                        emit_proj_fillers(w + 2)
                    emit_attention(w)
                    flush()
                emit_oproj_fillers(W512 - 1)
                flush()
            else:
                # all K/V (and Q) must exist before any attention window
                emit_xtw_load(3)
                for w in range(2, W512):
                    emit_proj_fillers(w)
                    flush()
                for w in range(W512):
                    emit_xn(w)
                    if w > 0:
                        emit_oproj_fillers(w - 1)
                    emit_attention(w)
                    flush()
                emit_oproj_fillers(W512 - 1)
                flush()
    return nc


def _host_prep(x, rms_w, Wq, Wk, Wv, Wo):
    import ml_dtypes
    f32 = np.float32
    bf16 = ml_dtypes.bfloat16
    x = np.asarray(x, f32)
    rms_w = np.asarray(rms_w, f32)
    wq_full = (np.asarray(Wq, f32) * rms_w[:, None] / math.sqrt(HD)).astype(f32)
    wk_full = (np.asarray(Wk, f32) * rms_w[:, None]).astype(f32)
    wv_full = (np.asarray(Wv, f32) * rms_w[:, None]).astype(f32)
    Wo = np.asarray(Wo, f32)

    inv_f = (1.0 / (10000.0 ** (np.arange(0, HD, 2, dtype=f32) / HD))).astype(f32)
    freqs = np.arange(S, dtype=f32)[:, None] * inv_f[None, :]   # [S, 32]
    cos = np.cos(freqs).astype(f32).T                           # [32, S]
    sin = np.sin(freqs).astype(f32).T
    cosT = np.tile(np.concatenate([cos, cos], 0), (2, 1))       # [128, S]
    sinT = np.tile(np.concatenate([-sin, sin], 0), (2, 1))

    kk = np.arange(128)[:, None]
    jj = np.arange(896)[None, :]
    maskb = (jj >= kk + 384).astype(f32)
    diag = np.eye(128, dtype=f32)

    per_core = []
    for c in range(NC):
        b, g = c // 4, c % 4
        heads = [8 * g + h for h in PERM]
        wq_g = np.ascontiguousarray(
            np.concatenate([wq_full[:, 64 * h : 64 * (h + 1)] for h in heads], axis=1))
        wo_g = np.ascontiguousarray(
            np.concatenate([Wo[64 * h : 64 * (h + 1), :] for h in heads], axis=0))
        wk_g = np.ascontiguousarray(wk_full[:, 128 * g : 128 * (g + 1)])
        wv_g = np.ascontiguousarray(wv_full[:, 128 * g : 128 * (g + 1)])
        # chunk-major resident layouts: [128, chunk-index * cols]
        wq_r = np.ascontiguousarray(
            wq_g.reshape(NDC, 128, CQ).transpose(1, 0, 2).reshape(128, NDC * CQ))
        wk_r = np.ascontiguousarray(
            wk_g.reshape(NDC, 128, 128).transpose(1, 0, 2).reshape(128, NDC * 128))
        wv_r = np.ascontiguousarray(
            wv_g.reshape(NDC, 128, 128).transpose(1, 0, 2).reshape(128, NDC * 128))
        wo_r = np.ascontiguousarray(
            wo_g.reshape(4, 128, D).transpose(1, 0, 2).reshape(128, 4 * D))
        rmsw_g = rms_w if g == 0 else np.zeros((D,), f32)
        rmswT = np.ascontiguousarray(
            rmsw_g.reshape(NDC, 128).T.astype(f32))        # [128, NDC]
        xb = x[b].astype(bf16)
        # window-major, chunk-major-per-partition swizzle of x^T (matches
        # the [p, (c t)] SBUF tile layout with one contiguous run/partition)
        xT2 = np.ascontiguousarray(
            xb.T.reshape(NDC, 128, W512, 512).transpose(1, 2, 0, 3)
                .reshape(128, W512 * NDC * 512))
        per_core.append({
            "xT": xT2,
            "xb": np.ascontiguousarray(xb),
            "wq": wq_r.astype(bf16), "wk": wk_r.astype(bf16),
            "wv": wv_r.astype(bf16), "wo": wo_r.astype(bf16),
            "cosT": np.ascontiguousarray(cosT.astype(bf16)),
            "sinT": np.ascontiguousarray(sinT.astype(bf16)),
            "maskb": maskb.astype(bf16), "rmsw": rmswT,
            "diag": diag.astype(bf16),
        })
    return per_core


def kernel(x, rms_w, Wq, Wk, Wv, Wo, apply_causal_mask, _trace=False):
    from concourse import bass_utils
    _install_patch()
    causal = bool(int(np.asarray(apply_causal_mask)))
    if causal not in _cache:
        _cache[causal] = _build(causal)
    nc = _cache[causal]
    in_maps = _host_prep(x, rms_w, Wq, Wk, Wv, Wo)
    r = bass_utils.run_bass_kernel_spmd(nc, in_maps, core_ids=list(range(NC)),
                                        trace=_trace)
    outs = [np.asarray(r.results[c]["out"], dtype=np.float32) for c in range(NC)]

    def unswizzle(o):
        # [p, (w c t)] -> [D, S] -> transpose to [S, D]
        return (o.reshape(128, W512, NDC, 512).transpose(2, 0, 1, 3)
                 .reshape(D, S).T)

    full = np.stack(
        [unswizzle(outs[4 * b] + outs[4 * b + 1] + outs[4 * b + 2]
                   + outs[4 * b + 3])
         for b in range(B)]).astype(np.float32)
    if _trace:
        kernel.last_exec_time_ns = r.exec_time_ns
        kernel.last_result = r
    return full


# revision 63
# speedup vs baseline: 1.1819x; 1.1710x over previous
"""GQA dense-transformer block (RMSNorm + QKV + RoPE + causal GQA attention
+ o_proj + residual) on 8 trn2 NeuronCores.

Sharding: 2 (batch) x 4 (head-group tensor parallel). Core c = 4*b + g handles
batch b, q-heads 8g..8g+7, kv-heads 2g..2g+1. Each core produces a partial
o_proj output (feature-major [D, S], bf16); the host sums the 4 partials per
batch and transposes. The RMS-normed residual is included on g==0 cores via
rmsw; g!=0 cores get rmsw=0.

v2 pipeline (vs baseline):
  - per-token scale s computed with one fused DVE sum-of-squares per token
    tile; broadcast across partitions with a diag*s -> ones-matmul trick
    (no DRAM bounce). All Sqrt activations run in the prologue so the ACT
    table set switches exactly once (Sqrt -> Exp).
  - xn = x^T * s computed once (in-place over the x^T tile); feeds Q/K/V
    projections AND the o_proj residual (affine_then_add fuses the
    rms_w-scale + PSUM add).
  - softmax normalize: PV pair accumulates into one [65,1024] PSUM tile;
    denominator row inverted with reciprocal_approx_fast (5x).
  - scheduling: o_proj(w-1) and proj(w+1) matmuls are interleaved into
    attention(w)'s kt loop as "fillers" so the PE never head-of-line
    blocks on the ACT exp chain and stays HAM-warm. PV runs with a
    1-iteration lag behind exp.
"""

import math
import numpy as np

# model dims (hardcoded per contract)
B, S, D = 2, 2048, 2048
HQ, HKV, HD = 32, 8, 64
NC = 8
NG = 4            # head groups
QH = 8            # q heads per core
KH = 2            # kv heads per core
CQ = QH * HD      # 512 q cols per core
W512 = S // 512   # 4 token windows
NT = S // 128     # 16 token tiles
NDC = D // 128    # 16 contraction chunks
PERM = [0, 4, 1, 5, 2, 6, 3, 7]  # local head order: ptile p = (h=p | h=p+4)

_cache = {}
_patched = [False]


def _legalize_bir_bytes(bir):
    """Walrus in this container accepts at most ONE embedded sem-wait per TPB
    instruction ("Too many sync wait commands"). Tile emits several when an
    instruction depends on multiple DMA queues. Split the extras into
    standalone EventSemaphore (pure-wait) instructions on the same engine
    immediately before the instruction — identical blocking semantics."""
    import json
    d = json.loads(bir if isinstance(bir, str) else bir.decode())
    n_split = 0
    stack = [d]
    while stack:
        o = stack.pop()
        if isinstance(o, dict):
            insts = o.get("instructions")
            if isinstance(insts, list) and insts and isinstance(insts[0], dict) \
               and "opcode" in insts[0]:
                new = []
                for inst in insts:
                    si = inst.get("sync_info") or {}
                    ws = si.get("on_wait") or []
                    if len(ws) > 1 and isinstance(inst.get("opcode"), str) \
                       and inst.get("opcode") not in (
                            "EventSemaphore", "UnconditionalBranch",
                            "Call", "ISA"):
                        for k, w in enumerate(ws[:-1]):
                            n_split += 1
                            new.append({
                                "debug": inst.get("debug", 0),
                                "engine": inst["engine"],
                                "ins": [], "outs": [],
                                "name": f"lw{n_split}_{inst['name']}",
                                "opcode": "EventSemaphore",
                                "sync_info": {"on_update": [], "on_wait": [w]},
                            })
                        si["on_wait"] = [ws[-1]]
                    new.append(inst)
                o["instructions"] = new
            else:
                stack.extend(o.values())
        elif isinstance(o, list):
            stack.extend(o)
    return json.dumps(d).encode()


def _install_patch():
    if _patched[0]:
        return
    from concourse import bass_utils as bu
    from concourse import bass2jax as b2j
    orig = bu.compile_bir_kernel

    def patched(bir, *a, **k):
        return orig(_legalize_bir_bytes(bir), *a, **k)

    bu.compile_bir_kernel = patched
    b2j.compile_bir_kernel = patched
    _patched[0] = True


def _build(causal: bool):
    import concourse.bass as bass
    import concourse.mybir as mybir
    from concourse.tile import TileContext

    fp32 = mybir.dt.float32
    bf16 = mybir.dt.bfloat16
    AF = mybir.ActivationFunctionType
    ALU = mybir.AluOpType

    nc = bass.Bass("TRN2")
    # x^T and out are stored window-major + chunk-major per partition so the
    # per-window DMA is one contiguous 16KB run per partition (128
    # descriptors instead of 2048)
    xT = nc.dram_tensor("xT", [128, W512 * NDC * 512], bf16,
                        kind="ExternalInput")
    xb_d = nc.dram_tensor("xb", [128, NT * D], bf16, kind="ExternalInput")
    wq_d = nc.dram_tensor("wq", [128, NDC * CQ], bf16, kind="ExternalInput")
    wk_d = nc.dram_tensor("wk", [128, NDC * 128], bf16, kind="ExternalInput")
    wv_d = nc.dram_tensor("wv", [128, NDC * 128], bf16, kind="ExternalInput")
    wo_d = nc.dram_tensor("wo", [128, 4 * D], bf16, kind="ExternalInput")
    cos_d = nc.dram_tensor("cosT", [128, S], bf16, kind="ExternalInput")
    sin_d = nc.dram_tensor("sinT", [128, S], bf16, kind="ExternalInput")
    maskb_d = nc.dram_tensor("maskb", [128, 896], bf16, kind="ExternalInput")
    rmsw_d = nc.dram_tensor("rmsw", [128, NDC], fp32, kind="ExternalInput")
    diag_d = nc.dram_tensor("diag", [128, 128], bf16, kind="ExternalInput")
    out = nc.dram_tensor("out", [128, W512 * NDC * 512], bf16,
                         kind="ExternalOutput")

    with TileContext(nc) as tc:
        with (
            nc.allow_low_precision(reason="bf16 pipeline within 2e-2 tolerance"),
            tc.tile_pool(name="res", bufs=1) as res,
            tc.tile_pool(name="dram", bufs=2, space="DRAM") as dpool,
            tc.tile_pool(name="xn_p", bufs=4) as xn_p,
            tc.tile_pool(name="obw_p", bufs=(2 if causal else 1)) as obw_p,
            tc.tile_pool(name="stat", bufs=1) as sp,
            tc.tile_pool(name="se_p", bufs=1) as se_p,
            tc.tile_pool(name="qt_p", bufs=(3 if causal else 4)) as qt_p,
            tc.tile_pool(name="at_p", bufs=2) as at_p,
            tc.tile_pool(name="rtmp", bufs=1) as rtmp,
            tc.tile_pool(name="bcs_p", bufs=2) as bcs_p,
            tc.tile_pool(name="pvc_p", bufs=2) as pvc_p,
            tc.tile_pool(name="aex", bufs=3) as aex,
            tc.tile_pool(name="inv_p", bufs=1) as inv_p,
            # PSUM: psc 2x2 banks + ppv 2 + pacc 2 = 8
            tc.tile_pool(name="psc", bufs=2, space="PSUM") as psc,
            tc.tile_pool(name="ppv", bufs=1, space="PSUM") as ppv,
            tc.tile_pool(name="pacc", bufs=2, space="PSUM") as pacc,
        ):
            # ---- resident tiles ----
            KTS = [res.tile([128, 512], bf16, tag=f"kts{w}", name=f"kts{w}")
                   for w in range(W512)]
            VAL = [res.tile([128, 4 * 130], bf16, tag=f"vall{w}", name=f"vall{w}")
                   for w in range(W512)]
            SBC = [res.tile([128, 512], bf16, tag=f"sbc{w}", name=f"sbc{w}")
                   for w in range(W512)]
            cosb = res.tile([128, S], bf16, tag="cosb", name="cosb")
            sinb = res.tile([128, S], bf16, tag="sinb", name="sinb")
            maskb = res.tile([128, 896], bf16, tag="maskb", name="maskb")
            rmswT = res.tile([128, NDC], fp32, tag="rmsw", name="rmswT")
            diag_b = res.tile([128, 128], bf16, tag="diag", name="diag_b")
            wq_r = res.tile([128, NDC * CQ], bf16, tag="wqr", name="wq_r")
            wk_r = res.tile([128, NDC * 128], bf16, tag="wkr", name="wk_r")
            wv_r = res.tile([128, NDC * 128], bf16, tag="wvr", name="wv_r")
            wo_r = res.tile([128, 4 * D], bf16, tag="wor", name="wo_r")
            ones1b = res.tile([1, 128], bf16, tag="ones1b", name="ones1b")
            ones128 = res.tile([128, 128], bf16, tag="ones128", name="ones128")
            epst = res.tile([128, 1], fp32, tag="epst", name="epst")
            ssq = res.tile([128, NT], fp32, tag="ssq", name="ssq")
            sq_all = res.tile([128, NT], fp32, tag="sq", name="sq_all")
            s_all = res.tile([128, NT], fp32, tag="sall", name="s_all")

            nc.vector.memset(ones1b[:, :], 1.0)
            nc.vector.memset(ones128[:, :], 1.0)
            nc.vector.memset(epst[:, :], float(np.finfo(np.float32).eps))
            for w in range(W512):
                nc.vector.memset(VAL[w][:, :], 1.0)
            # DMA issue order matters: queues are FIFO, and the first PE op
            # (psb(0)) transitively needs the stats tiles of window 0 — so
            # those xb loads and the small tables go first; the big x^T /
            # weight walls go after.
            nc.gpsimd.dma_start(out=diag_b[:, :], in_=diag_d[:, :])
            nc.gpsimd.dma_start(out=rmswT[:, :], in_=rmsw_d[:, :])

            xns = {}      # w -> xn tile ([128, NDC*512] bf16, in-place scaled)
            qts = {}      # (w, ct) -> QT tile
            ats = {}      # (w, p) -> AT tile
            fillers = []  # (is_pe, fn) queue

            def pump(n=1):
                # pop until n PE-emitting closures ran (aux closures are free)
                while n > 0 and fillers:
                    is_pe, fn = fillers.pop(0)
                    fn()
                    if is_pe:
                        n -= 1

            def flush():
                while fillers:
                    fillers.pop(0)[1]()

            # ---------------- x load / stats / xn ----------------
            def emit_xtw_load(w):
                t = xn_p.tile([128, NDC * 512], bf16, tag="xn", name="xn")
                nc.gpsimd.dma_start(
                    out=t[:, :],
                    in_=xT[:, NDC * 512 * w : NDC * 512 * (w + 1)])
                xns[w] = t

            def emit_stats_pair(k):
                # ssq[:, tt] = sum over features of x[token,:]^2 for the two
                # token tiles 2k, 2k+1. xb is host-swizzled so the pair is
                # one contiguous 8KB run per partition (128 descriptors).
                x_t = sp.tile([128, 2 * D], bf16, tag="xs", name="xs")
                nc.gpsimd.dma_start(out=x_t[:, :],
                                    in_=xb_d[:, 2 * D * k : 2 * D * (k + 1)])
                for j in range(2):
                    tt = 2 * k + j
                    xv = x_t[:, D * j : D * (j + 1)]
                    nc.vector.scalar_tensor_tensor(
                        out=xv, in0=xv, scalar=1.0, in1=xv,
                        op0=ALU.mult, op1=ALU.mult,
                        accum_out=ssq[:, tt : tt + 1])

            def emit_sbc(w, as_filler=False):
                # s = 1/sqrt(mean+eps) for the 4 token tiles of window w,
                # then broadcast across partitions: ones128^T @ (diag * s)
                def body():
                    tsl = slice(4 * w, 4 * (w + 1))
                    nc.scalar.activation(out=sq_all[:, tsl], in_=ssq[:, tsl],
                                         func=AF.Sqrt, bias=epst[:, 0:1],
                                         scale=1.0 / D)
                    nc.vector.reciprocal(out=s_all[:, tsl], in_=sq_all[:, tsl])
                    se = se_p.tile([128, 512], bf16, tag="se", name="se")
                    for vt in range(4):
                        nc.vector.tensor_scalar_mul(
                            se[:, 128 * vt : 128 * (vt + 1)], diag_b[:, :],
                            s_all[:, 4 * w + vt : 4 * w + vt + 1])
                    acc = pacc.tile([128, 512], fp32, tag="acc", name="psb")
                    nc.tensor.matmul(acc[:, :], ones128[:, :], se[:, :],
                                     start=True, stop=True)
                    nc.scalar.copy(out=SBC[w][:, :], in_=acc[:, :])
                if as_filler:
                    fillers.append((True, body))
                else:
                    body()

            def emit_xn(w):
                # xn = x^T * s for the o_proj residual only (projections
                # consume the raw tile). Emitted at iteration w: Tile's WAR
                # tracking orders the in-place writes after all proj reads,
                # and the idle Pool engine absorbs the 1x in-place cost.
                t = xns[w]
                for dc in range(NDC):
                    xsl = slice(512 * dc, 512 * (dc + 1))
                    nc.gpsimd.tensor_mul(t[:, xsl], t[:, xsl], SBC[w][:, :])

            # ---------------- projections + rope (filler-style) -------------
            def emit_rope(w, box, kind):
                # rotate-half RoPE straight off the proj PSUM tile (a
                # PSUM operand exempts the equal-base-partition rule the
                # cross-partition sin reads would otherwise violate).
                # The per-token rms scale s is folded in via sbc at the end
                # (projections consume RAW x^T).
                wsl = slice(512 * w, 512 * (w + 1))
                src = box["t"]
                tmp = rtmp.tile([128, 512], bf16, tag="rt", name="rt")
                for a, bidx in ((0, 1), (1, 0), (2, 3), (3, 2)):
                    nc.vector.tensor_mul(tmp[32 * a : 32 * (a + 1), :],
                                         src[32 * bidx : 32 * (bidx + 1), :],
                                         sinb[32 * a : 32 * (a + 1), wsl])
                t2 = rtmp.tile([128, 512], bf16, tag="rt2", name="rt2")
                nc.vector.tensor_mul(t2[:, :], src[:, :], cosb[:, wsl])
                nc.vector.tensor_add(tmp[:, :], tmp[:, :], t2[:, :])
                if kind == "K":
                    dst = KTS[w]
                else:
                    dst = qt_p.tile([128, 512], bf16, tag=f"qt{kind[1]}",
                                    name=f"qt{kind[1]}")
                    qts[(w, kind[1])] = dst
                nc.vector.tensor_mul(dst[:, :], tmp[:, :], SBC[w][:, :])

            def emit_vall(w, box):
                # V tokens sit on partitions here, so the per-token scale s
                # rides along as the activation's per-partition scale
                vs = box["t"]
                for vt in range(4):
                    nc.scalar.activation(
                        out=VAL[w][:, 130 * vt : 130 * vt + 130]
                            .rearrange("p (h x) -> p h x", h=2)[:, :, 0:64],
                        in_=vs[:, 128 * vt : 128 * (vt + 1)]
                            .rearrange("p (h d) -> p h d", h=2),
                        func=AF.Copy,
                        scale=s_all[:, 4 * w + vt : 4 * w + vt + 1])

            def emit_proj_fillers(w):
                xt = xns[w]
                # K pass: 4 fillers x 4 dc
                kbox = {}
                for blk in range(4):
                    def fk(blk=blk):
                        if blk == 0:
                            kbox["t"] = pacc.tile([128, 512], fp32, tag="acc",
                                                  name="ks")
                        for dc in range(4 * blk, 4 * blk + 4):
                            nc.tensor.matmul(
                                kbox["t"][:, :],
                                wk_r[:, dc * 128 : (dc + 1) * 128],
                                xt[:, 512 * dc : 512 * (dc + 1)],
                                start=(dc == 0), stop=(dc == NDC - 1))
                    fillers.append((True, fk))
                fillers.append((False, lambda: emit_rope(w, kbox, "K")))
                # V pass: 8 fillers x (2 dc x 4 vt)
                vbox = {}
                for blk in range(8):
                    def fv(blk=blk):
                        if blk == 0:
                            vbox["t"] = pacc.tile([128, 512], fp32, tag="acc",
                                                  name="vs")
                        for dc in range(2 * blk, 2 * blk + 2):
                            for vt in range(4):
                                nc.tensor.matmul(
                                    vbox["t"][:, 128 * vt : 128 * (vt + 1)],
                                    xt[:, 512 * dc + 128 * vt : 512 * dc + 128 * (vt + 1)],
                                    wv_r[:, dc * 128 : (dc + 1) * 128],
                                    start=(dc == 0 and vt == 0),
                                    stop=(dc == NDC - 1),
                                    skip_group_check=True)
                    fillers.append((True, fv))
                fillers.append((False, lambda: emit_vall(w, vbox)))
                # Q passes: 4 ct x (4 fillers x 4 dc)
                for ct in range(4):
                    qbox = {}
                    for blk in range(4):
                        def fq(ct=ct, blk=blk, qbox=qbox):
                            if blk == 0:
                                qbox["t"] = pacc.tile([128, 512], fp32,
                                                      tag="acc", name="qs")
                            for dc in range(4 * blk, 4 * blk + 4):
                                nc.tensor.matmul(
                                    qbox["t"][:, :],
                                    wq_r[:, dc * CQ + ct * 128 : dc * CQ + (ct + 1) * 128],
                                    xt[:, 512 * dc : 512 * (dc + 1)],
                                    start=(dc == 0), stop=(dc == NDC - 1))
                        fillers.append((True, fq))
                    fillers.append(
                        (False, lambda w=w, qbox=qbox, ct=ct:
                            emit_rope(w, qbox, ("Q", ct))))

            # ---------------- o_proj + residual (filler-style) --------------
            def emit_oproj_fillers(w):
                obw = obw_p.tile([128, NDC * 512], bf16, tag="obw", name="obw")
                wsl = slice(512 * w, 512 * (w + 1))
                for dc in range(NDC):
                    def fo(dc=dc):
                        pso = pacc.tile([128, 512], fp32, tag="acc", name="pso")
                        for c in range(4):
                            nc.tensor.matmul(
                                pso[:, :],
                                wo_r[:, c * D + dc * 128 : c * D + (dc + 1) * 128],
                                ats[(w, c)][:, :], start=(c == 0), stop=(c == 3))
                        xsl = slice(512 * dc, 512 * (dc + 1))
                        # obw = xn*rmsw + pso in one DVE op
                        nc.vector.scalar_tensor_tensor(
                            out=obw[:, xsl], in0=xns[w][:, xsl],
                            scalar=rmswT[:, dc : dc + 1], in1=pso[:, :],
                            op0=ALU.mult, op1=ALU.add)
                        if dc == NDC - 1:
                            nc.gpsimd.dma_start(
                                out=out[:, NDC * 512 * w : NDC * 512 * (w + 1)],
                                in_=obw[:, :])
                    fillers.append((True, fo))

            # ---------------- attention ----------------
            def emit_attention(w):
                kt_max = 4 * (w + 1) if causal else NT
                pending_norm = [None]
                for p in range(4):
                    at_t = at_p.tile([128, 512], bf16, tag=f"at{p}",
                                     name=f"at{p}")
                    ats[(w, p)] = at_t
                    qt_t = qts[(w, p)]
                    pvs = ppv.tile([65, 1024], fp32, tag="pv", name="pv")
                    exq = {}

                    def pv_pair(kt, pvs=pvs, exq=exq, kt_max=kt_max):
                        ex2 = exq.pop(kt)
                        wv_, lt = kt // 4, kt % 4
                        for h in range(2):
                            nc.tensor.matmul(
                                pvs[:, 512 * h : 512 * (h + 1)],
                                VAL[wv_][:, 130 * lt + 65 * h : 130 * lt + 65 * (h + 1)],
                                ex2[:, 512 * h : 512 * (h + 1)],
                                start=(kt == 0), stop=(kt == kt_max - 1),
                                skip_group_check=True)

                    for kt in range(kt_max):
                        dd = 128 * kt - 512 * w
                        sc2 = psc.tile([128, 1024], fp32, tag="sc", name="sc2")
                        for h in range(2):
                            nc.tensor.matmul(
                                sc2[:, 512 * h : 512 * (h + 1)],
                                KTS[kt // 4][64 * h : 64 * (h + 1),
                                             (kt % 4) * 128 : (kt % 4 + 1) * 128],
                                qt_t[64 * h : 64 * (h + 1), :],
                                start=True, stop=True)
                        ex2 = aex.tile([128, 1024], bf16, tag="ex", name="ex")
                        nc.scalar.activation(out=ex2[:, :], in_=sc2[:, :],
                                             func=AF.Exp)
                        if causal and 0 <= dd <= 384:
                            off = 384 - dd
                            ex2v = ex2[:, :].rearrange("p (a b) -> p a b", a=2)
                            mrep = maskb[:, off : off + 512].rearrange(
                                "p (a f) -> p a f", a=1).to_broadcast((128, 2, 512))
                            nc.vector.tensor_mul(ex2v, ex2v, mrep)
                        exq[kt] = ex2
                        pump(1)
                        if kt == 3 and pending_norm[0] is not None:
                            # previous p's softmax normalize: ~4 kt-cycles
                            # after its last PV, so the (slow) DVE
                            # reciprocal has drained and the bc matmuls
                            # don't stall the PE. PV(p) is unaffected — it
                            # accumulates into its own ppv slot (bufs=2).
                            pending_norm[0]()
                            pending_norm[0] = None
                        if kt >= 1:
                            pv_pair(kt - 1)
                    pump(1)
                    pv_pair(kt_max - 1)
                    # evacuate PV out of PSUM immediately (partition-packed
                    # to match AT) so the single ppv slot frees for the next
                    # p, then normalize from SBUF
                    pvc = pvc_p.tile([128, 512], bf16, tag="pvc", name="pvc")
                    for h in range(2):
                        nc.vector.tensor_copy(
                            out=pvc[64 * h : 64 * (h + 1), :],
                            in_=pvs[0:64, 512 * h : 512 * (h + 1)])
                    # 1/denominator: DVE reciprocal is 8 cyc/elem/lane, so a
                    # [1,1024] row costs 5.3us — bounce through DRAM to
                    # spread across 64 partitions ([64,16] => ~0.4us), then
                    # DMA-broadcast the result. Latency is hidden by the
                    # kt==3 deferral of norm_tail.
                    pvrow = inv_p.tile([1, 1024], fp32, tag="pvrow",
                                       name="pvrow")
                    nc.scalar.copy(out=pvrow[:, :], in_=pvs[64:65, :])
                    den_d = dpool.tile([1, 1024], fp32, tag="dend",
                                       name="den_d")
                    nc.gpsimd.dma_start(out=den_d[0:1, :], in_=pvrow[0:1, :])
                    dsp = inv_p.tile([64, 16], fp32, tag="dsp", name="dsp")
                    nc.gpsimd.dma_start(
                        out=dsp[:, :],
                        in_=den_d[0:1, :].rearrange("one (p c) -> (one p) c",
                                                    p=64))
                    ispb = inv_p.tile([64, 16], bf16, tag="ispb", name="ispb")
                    nc.vector.reciprocal(out=ispb[:, :], in_=dsp[:, :])
                    # plain write AP (rearranged DRAM write APs have been
                    # seen to race with their readers); reads below use a
                    # row view of the same [64,16] region (t = 16p + c)
                    inv_d = dpool.tile([64, 16], bf16, tag="invd",
                                       name="inv_d")
                    nc.gpsimd.dma_start(out=inv_d[:, :], in_=ispb[:, :])
                    inv_row = inv_d[:, :].rearrange("p c -> (p c)") \
                        .rearrange("(one t) -> one t", one=1)
                    # broadcast 1/d across partitions straight from DRAM
                    # (stride-0 source) — no PE matmul, no PSUM tile
                    bcs = bcs_p.tile([128, 512], bf16, tag="bcs", name="bcs")
                    for h in range(2):
                        nc.gpsimd.dma_start(
                            out=bcs[64 * h : 64 * (h + 1), :],
                            in_=inv_row[0:1, 512 * h : 512 * (h + 1)]
                                .to_broadcast((64, 512)))

                    def norm_tail(pvc=pvc, bcs=bcs, at_t=at_t):
                        nc.vector.tensor_mul(at_t[:, :], pvc[:, :], bcs[:, :])

                    pending_norm[0] = norm_tail
                if pending_norm[0] is not None:
                    pump(3)
                    pending_norm[0]()
                    pending_norm[0] = None

            # ---------------- schedule ----------------
            # prologue: stats for w0/w1 + both their proj passes run dense,
            # so the attention loop is always two windows ahead on proj.
            emit_stats_pair(0)
            emit_stats_pair(1)
            nc.gpsimd.dma_start(out=wk_r[:, :], in_=wk_d[:, :])
            nc.gpsimd.dma_start(out=wv_r[:, :], in_=wv_d[:, :])
            nc.gpsimd.dma_start(out=cosb[:, :], in_=cos_d[:, :])
            nc.gpsimd.dma_start(out=sinb[:, :], in_=sin_d[:, :])
            emit_xtw_load(0)
            emit_xtw_load(1)
            nc.gpsimd.dma_start(out=wq_r[:, :], in_=wq_d[:, :])
            emit_stats_pair(2)
            emit_stats_pair(3)
            # sbc(0)'s psb matmul would head-of-line-block the PE on the
            # stats chain if emitted first — slot it after the K-pass MMs
            # (and before the K rope, which reads SBC[0])
            mark = len(fillers)
            emit_proj_fillers(0)
            seg = fillers[mark:]
            del fillers[mark:]
            fillers.extend(seg[0:4])
            fillers.append((True, lambda: emit_sbc(0)))
            fillers.extend(seg[4:])
            flush()
            emit_sbc(1)
            # remaining stats + ALL Sqrt activations must complete in the
            # prologue: a Sqrt pumped mid-attention lands after the Exp
            # table set is resident and computes garbage
            for k in range(4, 8):
                emit_stats_pair(k)
            emit_proj_fillers(1)
            flush()
            emit_sbc(2)
            emit_sbc(3)
            nc.gpsimd.dma_start(out=wo_r[:, :], in_=wo_d[:, :])
            nc.gpsimd.dma_start(out=maskb[:, :], in_=maskb_d[:, :])
            emit_xtw_load(2)

            if causal:
                for w in range(W512):
                    if w == 0:
                        emit_xtw_load(3)
                    emit_xn(w)
                    if w > 0:
                        emit_oproj_fillers(w - 1)
                    if w + 2 < W512:
                        emit_proj_fillers(w + 2)
                    emit_attention(w)
                    flush()
                emit_oproj_fillers(W512 - 1)
                flush()
            else:
                # all K/V (and Q) must exist before any attention window
                emit_xtw_load(3)
                for w in range(2, W512):
                    emit_proj_fillers(w)
                    flush()
                for w in range(W512):
                    emit_xn(w)
                    if w > 0:
                        emit_oproj_fillers(w - 1)
                    emit_attention(w)
                    flush()
                emit_oproj_fillers(W512 - 1)
                flush()
    return nc


def _host_prep(x, rms_w, Wq, Wk, Wv, Wo):
    import ml_dtypes
    f32 = np.float32
    bf16 = ml_dtypes.bfloat16
    x = np.asarray(x, f32)
    rms_w = np.asarray(rms_w, f32)
    wq_full = (np.asarray(Wq, f32) * rms_w[:, None] / math.sqrt(HD)).astype(f32)
    wk_full = (np.asarray(Wk, f32) * rms_w[:, None]).astype(f32)
    wv_full = (np.asarray(Wv, f32) * rms_w[:, None]).astype(f32)
    Wo = np.asarray(Wo, f32)

    inv_f = (1.0 / (10000.0 ** (np.arange(0, HD, 2, dtype=f32) / HD))).astype(f32)
    freqs = np.arange(S, dtype=f32)[:, None] * inv_f[None, :]   # [S, 32]
    cos = np.cos(freqs).astype(f32).T                           # [32, S]
    sin = np.sin(freqs).astype(f32).T
    cosT = np.tile(np.concatenate([cos, cos], 0), (2, 1))       # [128, S]
    sinT = np.tile(np.concatenate([-sin, sin], 0), (2, 1))

    kk = np.arange(128)[:, None]
    jj = np.arange(896)[None, :]
    maskb = (jj >= kk + 384).astype(f32)
    diag = np.eye(128, dtype=f32)

    per_core = []
    for c in range(NC):
        b, g = c // 4, c % 4
        heads = [8 * g + h for h in PERM]
        wq_g = np.ascontiguousarray(
            np.concatenate([wq_full[:, 64 * h : 64 * (h + 1)] for h in heads], axis=1))
        wo_g = np.ascontiguousarray(
            np.concatenate([Wo[64 * h : 64 * (h + 1), :] for h in heads], axis=0))
        wk_g = np.ascontiguousarray(wk_full[:, 128 * g : 128 * (g + 1)])
        wv_g = np.ascontiguousarray(wv_full[:, 128 * g : 128 * (g + 1)])
        # chunk-major resident layouts: [128, chunk-index * cols]
        wq_r = np.ascontiguousarray(
            wq_g.reshape(NDC, 128, CQ).transpose(1, 0, 2).reshape(128, NDC * CQ))
        wk_r = np.ascontiguousarray(
            wk_g.reshape(NDC, 128, 128).transpose(1, 0, 2).reshape(128, NDC * 128))
        wv_r = np.ascontiguousarray(
            wv_g.reshape(NDC, 128, 128).transpose(1, 0, 2).reshape(128, NDC * 128))
        wo_r = np.ascontiguousarray(
            wo_g.reshape(4, 128, D).transpose(1, 0, 2).reshape(128, 4 * D))
        rmsw_g = rms_w if g == 0 else np.zeros((D,), f32)
        rmswT = np.ascontiguousarray(
            rmsw_g.reshape(NDC, 128).T.astype(f32))        # [128, NDC]
        xb = x[b].astype(bf16)
        # window-major, chunk-major-per-partition swizzle of x^T (matches
        # the [p, (c t)] SBUF tile layout with one contiguous run/partition)
        xT2 = np.ascontiguousarray(
            xb.T.reshape(NDC, 128, W512, 512).transpose(1, 2, 0, 3)
                .reshape(128, W512 * NDC * 512))
        # token-tile-major swizzle of x for the stats loads: partition p
        # holds token-tile rows (128tt + p) back to back
        xb2 = np.ascontiguousarray(
            xb.reshape(NT, 128, D).transpose(1, 0, 2).reshape(128, NT * D))
        per_core.append({
            "xT": xT2,
            "xb": xb2,
            "wq": wq_r.astype(bf16), "wk": wk_r.astype(bf16),
            "wv": wv_r.astype(bf16), "wo": wo_r.astype(bf16),
            "cosT": np.ascontiguousarray(cosT.astype(bf16)),
            "sinT": np.ascontiguousarray(sinT.astype(bf16)),
            "maskb": maskb.astype(bf16), "rmsw": rmswT,
            "diag": diag.astype(bf16),
        })
    return per_core


def kernel(x, rms_w, Wq, Wk, Wv, Wo, apply_causal_mask, _trace=False):
    from concourse import bass_utils
    _install_patch()
    causal = bool(int(np.asarray(apply_causal_mask)))
    if causal not in _cache:
        _cache[causal] = _build(causal)
    nc = _cache[causal]
    in_maps = _host_prep(x, rms_w, Wq, Wk, Wv, Wo)
    r = bass_utils.run_bass_kernel_spmd(nc, in_maps, core_ids=list(range(NC)),
                                        trace=_trace)
    outs = [np.asarray(r.results[c]["out"], dtype=np.float32) for c in range(NC)]

    def unswizzle(o):
        # [p, (w c t)] -> [D, S] -> transpose to [S, D]
        return (o.reshape(128, W512, NDC, 512).transpose(2, 0, 1, 3)
                 .reshape(D, S).T)

    full = np.stack(
        [unswizzle(outs[4 * b] + outs[4 * b + 1] + outs[4 * b + 2]
                   + outs[4 * b + 3])
         for b in range(B)]).astype(np.float32)
    if _trace:
        kernel.last_exec_time_ns = r.exec_time_ns
        kernel.last_result = r
    return full
